# revision 30
# baseline (speedup 1.0000x reference)
import sys

if "/opt/trn_rl_repo" not in sys.path:
    sys.path.insert(0, "/opt/trn_rl_repo")

import numpy as np

import concourse.bass as bass
import concourse.bacc as bacc
import concourse.mybir as mybir
from concourse.masks import make_identity
from concourse.tile import TileContext

# Problem dims (hardcoded per contract)
B, CIN, COUT, F, N, K = 128, 16, 16, 512, 32, 2
NCORES = 8
BS = B // NCORES          # batch shard per core = 16
P = 128                   # partitions
FL = 4                    # f = fh*FL + fl, fh in [0,128), fl in [0,4)
NG = 4                    # node groups
GN = N // NG              # 8 nodes per group
U = GN * K                # 16 m-positions per group (u = 2*nl + k)
CM = COUT * N * K         # 1024 (c, m) columns
WN = COUT * U             # 256 matmul output columns per group

_nc_cache = None


def _build():
    """out[b,c,f,2n+k] = sum_i x[b,i,f,n]*Weff[n,i,c,k] + beff[n,c]  per core.

    f is split as (fh, fl) with fh on SBUF partitions so every DMA runs at
    fat-descriptor line rate. The contraction runs in bf16 (error budget is
    2e-2; bf16 matmul/transpose is 1 PE cycle/row vs 4 for fp32). Per b:
      - ONE load  x[b] (1MB, 512B descriptors) -> xin[fh, (i, fl, n)]
      - repack+convert f32->bf16 (DVE/ACT)     -> xrp[fh, (fl, g, i, nl)]
      - PE-transpose (bf16)                    -> pt_g[(i,nl), (fl, fh)]
      - ACT copy PSUM->SBUF                    -> xt_g
      - per fl: 4 bf16 matmuls (block-diag weights) -> pm[fh, (g, c, u)] f32
      - per fl: one fused DVE bias-add evac    -> sth[fh, (c, fll, m)]
      - TWO stores per b (1MB each, 512B descriptors, one per fl-pair),
        issued on the Pool queue (SWDGE) so their sem waits never block the
        SP queue dispatching loads.
    """
    nc = bacc.Bacc(dynamic_dma_scratch_size=65536)
    f32 = mybir.dt.float32
    bf16 = mybir.dt.bfloat16
    x = nc.declare_dram_parameter("x", [BS, CIN, F, N], f32, isOutput=False)
    # one packed const tensor (single DMA, single HWDGE slot), columns:
    #   [0:CM)            wc    (compact weights)   wc[i, col(g,c,u)]
    #   [CM:2CM)          msk   (block-diag mask, rows GN.. zero-padded)
    #   [2CM:2CM+P)       sel1  (partition selector p//GN == i)
    #   [2CM+P:2CM+2P)    sel2  (partition selector p%GN == nl, rows GN.. zero)
    #   [2CM+2P:3CM+2P)   bias  (row 0 only)
    CPK = 3 * CM + 2 * P
    cpkp = nc.declare_dram_parameter("cpack", [CIN, CPK], bf16, isOutput=False)
    out = nc.declare_dram_parameter("out", [BS, COUT, F, N * K], f32, isOutput=True)

    with TileContext(nc) as tc:
        with (
            tc.tile_pool(name="const", bufs=1) as const,
            tc.tile_pool(name="xin", bufs=5) as xpool,
            tc.tile_pool(name="xrp", bufs=2) as rpool,
            tc.tile_pool(name="xt", bufs=8) as xtpool,
            tc.tile_pool(name="stage", bufs=2) as stpool,
            tc.tile_pool(name="pt", bufs=4, space="PSUM") as ptpool,
            tc.tile_pool(name="pm", bufs=2, space="PSUM") as pmpool,
        ):
            # ---- first load before const DMAs: the load stream is the
            # critical DMA-pool resource; consts are only needed ~10us in.
            def load_x(b):
                xin = xpool.tile([P, CIN * FL * N], f32)
                nc.sync.dma_start(
                    out=xin[:, :].rearrange("p (i fl n) -> p i fl n", i=CIN, fl=FL),
                    in_=x[b].rearrange("i (fh fl) n -> fh i fl n", fl=FL),
                )
                return xin

            xin_next = load_x(0)

            ident0 = const.tile([P, P], f32)
            make_identity(nc, ident0)
            # bf16 identity for bf16 transpose-mode Matmult; re-copy via
            # ScalarE so transposes depend on a single engine
            ident = const.tile([P, P], bf16, tag="ident2")
            nc.scalar.copy(out=ident[:], in_=ident0[:])
            # ---- single packed const load, then build the 256KB
            # block-diagonal weight table on-chip from the compact factors
            # (saves ~560ns on the bottleneck DMA pool):
            #   wt[(i,nl), col] = wc[i, col] * msk[nl, col],
            # each factor broadcast across partitions by a selector matmul.
            cpk = const.tile([CIN, CPK], bf16, tag="cpk")
            nc.sync.dma_start(out=cpk[:, :], in_=cpkp[:, :])
            O_WC, O_MSK, O_S1, O_S2, O_B = 0, CM, 2 * CM, 2 * CM + P, 2 * CM + 2 * P

            def bcast_matmul(dst, src_o, sel_o):
                for h in range(2):
                    # each 512-col half stays within one PSUM bank
                    nc.tensor.matmul(
                        dst[:, h * 512 : (h + 1) * 512],
                        cpk[:, sel_o : sel_o + P],
                        cpk[:, src_o + h * 512 : src_o + (h + 1) * 512],
                        start=True,
                        stop=True,
                    )

            # mask factor -> PSUM -> SBUF (TensorTensor may read only one
            # PSUM operand, so one factor must land in SBUF first)
            pwm = pmpool.tile([P, NG * WN], f32, tag="pm")
            bcast_matmul(pwm, O_MSK, O_S2)
            mska = const.tile([P, CM], bf16, tag="mska")
            nc.scalar.copy(out=mska[:], in_=pwm[:])
            # weight factor stays in PSUM; multiply into the SBUF table
            pwa = pmpool.tile([P, NG * WN], f32, tag="pm")
            bcast_matmul(pwa, O_WC, O_S1)
            wt = const.tile([P, NG * WN], bf16)
            nc.vector.tensor_mul(wt[:], pwa[:], mska[:])
            # bias: broadcast across partitions with a K=1 ones-matmul
            # instead of a 512KB broadcast DMA.
            ones1 = const.tile([1, P], bf16, tag="ones1")
            nc.vector.memset(ones1[0:1, :], 1.0)
            bt = const.tile([P, CM], f32)
            pb = pmpool.tile([P, NG * WN], f32, tag="pm")
            for h in range(2):
                # each 512-col half stays within one PSUM bank
                nc.tensor.matmul(
                    pb[:, h * 512 : (h + 1) * 512],
                    ones1[0:1, :],
                    cpk[0:1, O_B + h * 512 : O_B + (h + 1) * 512],
                    start=True,
                    stop=True,
                )
            nc.scalar.copy(out=bt[:], in_=pb[:])
            btv = bt[:, :].rearrange("p (g c u) -> p g c u", g=NG, c=COUT)

            for b in range(BS):
                # ---- load: xin[fh, i*128 + fl*32 + n] = x[b, i, fh*4+fl, n]
                xin = xin_next
                if b + 1 < BS:
                    xin_next = load_x(b + 1)

                # ---- repack (and f32->bf16 convert) so each (fl, g)
                # transpose input is contiguous (matmul stationary APs allow
                # only ONE free dim); one copy per (g, fl) so each transpose
                # waits on a single small copy:
                # xrp[fh, fl*512 + g*128 + i*8 + nl] = xin[fh, i*128 + fl*32 + g*8 + nl]
                xinv = xin[:, :].rearrange(
                    "p (i fl g nl) -> p i fl g nl", i=CIN, fl=FL, g=NG
                )
                xrp = rpool.tile([P, CIN * FL * N], bf16)
                xrpv = xrp[:, :].rearrange(
                    "p (fl g i nl) -> p fl g i nl", fl=FL, g=NG, i=CIN
                )
                for g in range(NG):
                    for fl in range(FL):
                        # dst contiguous 128 = (i, nl); src (i@128, nl@1)
                        if (g * FL + fl) % 2 == 0:
                            nc.vector.tensor_copy(
                                out=xrpv[:, fl, g], in_=xinv[:, :, fl, g, :]
                            )
                        else:
                            nc.scalar.copy(
                                out=xrpv[:, fl, g], in_=xinv[:, :, fl, g, :]
                            )

                # ---- transpose: xt_g[(i*8+nl), fl*128 + fh]  (bf16)
                xts = []
                for g in range(NG):
                    pt = ptpool.tile([P, FL * P], bf16)
                    for fl in range(FL):
                        nc.tensor.transpose(
                            pt[:, fl * P : (fl + 1) * P],
                            xrp[:, fl * 512 + g * P : fl * 512 + (g + 1) * P],
                            ident[:],
                        )
                    xt = xtpool.tile([P, FL * P], bf16)
                    nc.scalar.copy(out=xt[:], in_=pt[:])
                    xts.append(xt)

                # ---- per fl: 4 matmuls into one [P, 1024] f32 PSUM tile,
                # then a single fused bias-add evac into STB[fh, (c, fl, m)].
                # Stage per fl-PAIR so each half can store as soon as its two
                # evacs land (smooths the DMA drain phase).
                outv = out[b].rearrange(
                    "c (fh flh fll) m -> fh flh c fll m", flh=FL // 2, fll=2
                )
                for flh in range(FL // 2):
                    sth = stpool.tile([P, COUT * 2 * N * K], f32, tag=f"sth{flh}")
                    shv = sth[:, :].rearrange(
                        "p (c fll m) -> p c fll m", c=COUT, fll=2
                    )
                    for fll in range(2):
                        fl = flh * 2 + fll
                        pm = pmpool.tile([P, NG * WN], f32, tag="pm")
                        for g in range(NG):
                            nc.tensor.matmul(
                                pm[:, g * WN : (g + 1) * WN],
                                xts[g][:, fl * P : (fl + 1) * P],
                                wt[:, g * WN : (g + 1) * WN],
                                start=True,
                                stop=True,
                            )
                        # dst cols c*128 + fll*64 + g*16 + u, as (g, c, u)
                        dst = shv[:, :, fll, :].rearrange(
                            "p c (g u) -> p g c u", g=NG
                        )
                        nc.vector.tensor_add(
                            dst,
                            pm[:, :].rearrange("p (g c u) -> p g c u", g=NG, c=COUT),
                            btv,
                        )
                    # ---- store half: out[b, c, fh*4+flh*2+fll, m] <- sth
                    # (512B descriptors — still full DMA line rate). Pool
                    # (SWDGE) queue keeps store waits off the load queue.
                    nc.gpsimd.dma_start(
                        out=outv[:, flh],
                        in_=shv,
                    )
    nc.compile()
    return nc


def _fold_weights(W1, b1, W2, b2):
    # Weff[n,i,c,k] = sum_o W1[n,i,o,k] * W2[n,o,c]; beff[n,c] = b1[n]@W2[n] + b2[n]
    Weff = np.einsum("niok,noc->nick", W1, W2).astype(np.float32)
    beff = (np.einsum("no,noc->nc", b1, W2) + b2).astype(np.float32)

    # The on-chip block-diag table is bigw[(i,nl), col(g,c,u)] =
    # wc[i, col] * msk[nl, col]:
    #   wc[i, g*256 + c*16 + u] = Weff[g*8 + u//2, i, c, u%2]
    #   msk[nl, col]            = 1[u//2 == nl]
    gg, cc, uu = np.meshgrid(
        np.arange(NG), np.arange(COUT), np.arange(U), indexing="ij"
    )
    # packed const tensor [CIN, 3*CM + 2*P]: wc | msk | sel1 | sel2 | bias
    cpack = np.zeros((CIN, 3 * CM + 2 * P), np.float32)
    for i in range(CIN):
        cpack[i, 0:CM] = Weff[gg * GN + uu // 2, i, cc, uu % 2].reshape(-1)
    # msk rows GN.. stay zero (matched by zero rows in sel2)
    cpack[:GN, CM : 2 * CM] = (
        uu[None, :, :, :] // 2 == np.arange(GN)[:, None, None, None]
    ).reshape(GN, CM)
    cpack[:, 2 * CM : 2 * CM + P] = (
        np.arange(P)[None, :] // GN == np.arange(CIN)[:, None]
    )
    cpack[:GN, 2 * CM + P : 2 * CM + 2 * P] = (
        np.arange(P)[None, :] % GN == np.arange(GN)[:, None]
    )
    # bias_flat[g*256 + c*16 + u] = beff[g*8 + u//2, c], row 0 only
    cpack[0, 2 * CM + 2 * P :] = beff[gg * GN + uu // 2, cc].reshape(-1)
    return cpack


def kernel(x, W1, b1, W2, b2):
    global _nc_cache
    import ml_dtypes
    from concourse.bass_utils import run_bass_kernel_spmd

    x = np.ascontiguousarray(np.asarray(x, dtype=np.float32))
    cpack = _fold_weights(
        np.asarray(W1, np.float32),
        np.asarray(b1, np.float32),
        np.asarray(W2, np.float32),
        np.asarray(b2, np.float32),
    ).astype(ml_dtypes.bfloat16)
    if _nc_cache is None:
        _nc_cache = _build()
    nc = _nc_cache
    in_maps = [
        {"x": x[d * BS : (d + 1) * BS], "cpack": cpack}
        for d in range(NCORES)
    ]
    res = run_bass_kernel_spmd(nc, in_maps, list(range(NCORES)))
    return np.concatenate([res.results[d]["out"] for d in range(NCORES)], axis=0)
